# revision 7
# baseline (speedup 1.0000x reference)
"""LogGaborConv2d on 8 TRN2 NeuronCores.

Strategy: data-parallel over batch (8 images -> 8 cores). Per core:
- Gabor weights [O=128, I=64, 3, 3] computed on host (73k elements),
  uploaded as bf16 lhsT blocks, duplicated across both PE row groups.
- 3x3 conv as 9 accumulating bf16 matmuls (K=64 input channels) over a
  column-padded flat image stream (width 258), windows of 512 pixels
  into PSUM banks. bf16 streams at 1 col/cycle (2x the fp32 rate) and
  enables fast weight load.
- The 128 PE rows are split into two row-groups: partitions 0:64
  process the top half of the image, partitions 64:128 the bottom half,
  as concurrent K=64 matmuls (tile_position row groups), doubling PE
  throughput vs a single K=64 stream.
- The whole per-core input stream (8.7 MB bf16) lives in SBUF: one
  persistent tile filled by 8 chunked DMAs on the scalar HWDGE ring,
  so input prefetch never queues behind output stores (sync ring).
- Output staged as bf16 (PSUM fp32 -> SBUF bf16 copies), halving store
  traffic; host upcasts to fp32.
"""
import math

import numpy as np
import ml_dtypes

import concourse.bacc as bacc
import concourse.bass as bass  # noqa: F401
import concourse.mybir as mybir
import concourse.tile as tile
from concourse.bass_utils import run_bass_kernel_spmd

F32 = mybir.dt.float32
BF16 = mybir.dt.bfloat16
BF16_NP = np.dtype(ml_dtypes.bfloat16)

# problem constants
NB, C, H, W = 8, 64, 256, 256
O = 128
WP = W + 2            # padded row width
SL = (H + 2) * WP     # padded input stream length (incl. top/bottom pad rows)
OL = H * WP           # padded output stream length
NWIN = OL // 512      # 129 windows of 512
GUARD = 4             # leading guard zeros in the device stream
NW_A = 64             # windows 0:64 on partitions 0:64; 64:129 on 64:128
XB = 33808            # per-half device stream cols (>= 512*65 + 517 + 4 + 511)
# first chunk covers exactly window group 0 (cols <= 1544) so the first
# matmul can start as soon as ~0.4 MB lands; the rest stream in behind it
CHUNKS = [1552] + [4608] * 7  # sums to XB
DELTA = 0.001
_GRID = (-1.0, 0.5, 2.0)


def _taps():
    """(tap_index, ky, kx, stream_delta) for the 9 taps."""
    out = []
    for ky in range(3):
        for kx in range(3):
            out.append((3 * ky + kx, ky, kx, ky * WP + (kx - 1)))
    return out


def build_kernel():
    nc = bacc.Bacc("TRN2", target_bir_lowering=False)
    x = nc.dram_tensor("x", [2 * C, XB], BF16, kind="ExternalInput")
    w = nc.dram_tensor("w", [O, 1152], BF16, kind="ExternalInput")
    y = nc.dram_tensor("y", [O, OL], BF16, kind="ExternalOutput")

    taps = _taps()

    with tile.TileContext(nc) as tc:
        with (
            tc.tile_pool(name="wg", bufs=1) as wg,
            tc.tile_pool(name="outp", bufs=3) as outp,
            tc.tile_pool(name="ps", bufs=2, space="PSUM") as ps,
        ):
            wt = wg.tile([O, 1152], BF16)
            # wt rides the sync ring so it transfers in parallel with the
            # first input chunk on the scalar ring
            nc.sync.dma_start(wt[:], w[:])
            xs = wg.tile([2 * C, XB], BF16)
            # input chunks ride the gpsimd SWDGE ring: the scalar and sync
            # queues then carry no input-DMA waits that could block the
            # PSUM->SBUF copies behind them mid-stream
            c0 = 0
            for clen in CHUNKS:
                nc.gpsimd.dma_start(
                    xs[:, c0 : c0 + clen], x[:, c0 : c0 + clen]
                )
                c0 += clen

            # Warm-up: ~10 matmuls on a never-written scratch tile. No data
            # deps, so they issue right after the preamble barrier and bring
            # the PE HAM clock-gate to 8/8 while the first input chunk DMA
            # is still in flight. Results land in a throwaway PSUM tile that
            # tag-rotates into the real pool afterwards.
            dmy = outp.tile([O, 512], BF16, tag="dmy", name="dmy")
            nc.gpsimd.memset(dmy[:], 1.0)
            dmy_ps = ps.tile([O, 512], F32, tag="a0", name="dmy_ps")
            for _ in range(10):
                nc.tensor.matmul(
                    dmy_ps[:], dmy[0:O, 0:128], dmy[0:O, 0:512],
                    start=True, stop=True,
                )

            def emit_group(g, na, nb):
                pa = [
                    ps.tile([O, 512], F32, tag=f"a{j}", name=f"pa{j}")
                    for j in range(na)
                ]
                pb = [
                    ps.tile([O, 512], F32, tag=f"b{j}", name=f"pb{j}")
                    for j in range(nb)
                ]
                ntap = len(taps)
                for t, ky, kx, delta in taps:
                    lhs_a = wt[0:C, 128 * t : 128 * t + 128]
                    lhs_b = wt[C : 2 * C, 128 * t : 128 * t + 128]
                    first = t == 0
                    last = t == ntap - 1
                    for j in range(max(na, nb)):
                        o = 512 * (2 * g + j) + delta + GUARD
                        if j < na:
                            nc.tensor.matmul(
                                pa[j][:], lhs_a, xs[0:C, o : o + 512],
                                start=first, stop=last,
                            )
                        if j < nb:
                            nc.tensor.matmul(
                                pb[j][:], lhs_b, xs[C : 2 * C, o : o + 512],
                                start=first, stop=last,
                            )
                ot = outp.tile([O, 512 * (na + nb)], BF16, tag="ot", name="ot")
                for j in range(na):
                    if j % 2 == 0:
                        nc.scalar.copy(ot[:, 512 * j : 512 * j + 512], pa[j][:])
                    else:
                        nc.vector.tensor_copy(
                            ot[:, 512 * j : 512 * j + 512], pa[j][:]
                        )
                for j in range(nb):
                    c0 = 512 * (na + j)
                    if j % 2 == 1:
                        nc.scalar.copy(ot[:, c0 : c0 + 512], pb[j][:])
                    else:
                        nc.vector.tensor_copy(ot[:, c0 : c0 + 512], pb[j][:])
                if na:
                    nc.sync.dma_start(
                        y[:, 512 * 2 * g : 512 * (2 * g + na)],
                        ot[:, 0 : 512 * na],
                    )
                if nb:
                    nc.sync.dma_start(
                        y[:, 512 * (NW_A + 2 * g) : 512 * (NW_A + 2 * g + nb)],
                        ot[:, 512 * na : 512 * (na + nb)],
                    )

            for g in range(32):
                emit_group(g, 2, 2)
            # final window 128 (B-half local window 64)
            emit_group(32, 0, 1)

    nc.compile()
    return nc


_NC_CACHE = None


def _get_nc():
    global _NC_CACHE
    if _NC_CACHE is None:
        _NC_CACHE = build_kernel()
    return _NC_CACHE


def _host_weights(freq, theta, sigma, psi, f0, theta0, xg, yg):
    th = np.asarray(theta, np.float32)[:, :, None, None]
    sg = np.asarray(sigma, np.float32)[:, :, None, None]
    fr = np.asarray(freq, np.float32)[:, :, None, None]
    ps = np.asarray(psi, np.float32)[:, :, None, None]
    xg = np.asarray(xg, np.float32)
    yg = np.asarray(yg, np.float32)
    f0 = np.asarray(f0, np.float32)
    theta0 = np.asarray(theta0, np.float32)
    rotx = xg * np.cos(th) + yg * np.sin(th)
    roty = -xg * np.sin(th) + yg * np.cos(th)
    r = np.sqrt(rotx**2 + roty**2 + DELTA)
    g_rad = np.exp(-((np.log(r) - np.log(f0)) / (2.0 * np.log(sg / f0))) ** 2)
    g_ang = np.exp(-((th - theta0) ** 2) / (2.0 * sg**2))
    g = g_rad * g_ang * np.cos(fr * r + ps) / (2.0 * math.pi * sg**2)
    return g.astype(np.float32)  # [O, I, 3, 3]


def kernel(input_tensor, freq, theta, sigma, psi, f0, theta0, xg, yg):
    x = np.ascontiguousarray(np.asarray(input_tensor, dtype=np.float32))
    wfull = _host_weights(freq, theta, sigma, psi, f0, theta0, xg, yg)
    wt = np.zeros((O, 1152), np.float32)
    for t, ky, kx, _delta in _taps():
        blk = wfull[:, :, ky, kx].T  # lhsT [K=64, M=128]
        wt[0:C, 128 * t : 128 * t + 128] = blk
        wt[C : 2 * C, 128 * t : 128 * t + 128] = blk
    wt_bf = np.ascontiguousarray(wt.astype(BF16_NP))

    nc = _get_nc()
    in_maps = []
    for c in range(NB):
        s = np.zeros((C, H + 2, WP), np.float32)
        s[:, 1 : H + 1, 1 : W + 1] = x[c]
        s_bf = s.reshape(C, SL).astype(BF16_NP)
        xp = np.zeros((2 * C, XB), BF16_NP)
        xp[0:C, GUARD:XB] = s_bf[:, 0 : XB - GUARD]
        b0 = 512 * NW_A - GUARD
        nb_len = min(SL - b0, XB)
        xp[C : 2 * C, 0:nb_len] = s_bf[:, b0 : b0 + nb_len]
        in_maps.append({"x": xp, "w": wt_bf})
    res = run_bass_kernel_spmd(nc, in_maps, core_ids=list(range(NB)))
    out = np.empty((NB, O, H, W), np.float32)
    for c in range(NB):
        yv = np.asarray(res.results[c]["y"]).reshape(O, H, WP)
        out[c] = yv[:, :, 1 : W + 1].astype(np.float32)
    return out


# revision 8
# speedup vs baseline: 1.0283x; 1.0283x over previous
"""LogGaborConv2d on 8 TRN2 NeuronCores.

Strategy: data-parallel over batch (8 images -> 8 cores). Per core:
- Gabor weights [O=128, I=64, 3, 3] computed on host (73k elements),
  uploaded as bf16 lhsT blocks, duplicated across both PE row groups.
- 3x3 conv as 9 accumulating bf16 matmuls (K=64 input channels) over a
  column-padded flat image stream (width 258), windows of 512 pixels
  into PSUM banks. bf16 streams at 1 col/cycle (2x the fp32 rate) and
  enables fast weight load.
- The 128 PE rows are split into two row-groups: partitions 0:64
  process the top half of the image, partitions 64:128 the bottom half,
  as concurrent K=64 matmuls (tile_position row groups), doubling PE
  throughput vs a single K=64 stream.
- The whole per-core input stream (8.7 MB bf16) lives in SBUF: one
  persistent tile filled by 8 chunked DMAs on the scalar HWDGE ring,
  so input prefetch never queues behind output stores (sync ring).
- Output staged as bf16 (PSUM fp32 -> SBUF bf16 copies), halving store
  traffic; host upcasts to fp32.
"""
import math

import numpy as np
import ml_dtypes

import concourse.bacc as bacc
import concourse.bass as bass  # noqa: F401
import concourse.mybir as mybir
import concourse.tile as tile
from concourse.bass_utils import run_bass_kernel_spmd

F32 = mybir.dt.float32
BF16 = mybir.dt.bfloat16
BF16_NP = np.dtype(ml_dtypes.bfloat16)

# problem constants
NB, C, H, W = 8, 64, 256, 256
O = 128
WP = W + 2            # padded row width
SL = (H + 2) * WP     # padded input stream length (incl. top/bottom pad rows)
OL = H * WP           # padded output stream length
NWIN = OL // 512      # 129 windows of 512
GUARD = 4             # leading guard zeros in the device stream
NW_A = 64             # windows 0:64 on partitions 0:64; 64:129 on 64:128
XB = 33808            # per-half device stream cols (>= 512*65 + 517 + 4 + 511)
# first chunk covers exactly window group 0 (cols <= 1544) so the first
# matmul can start as soon as ~0.4 MB lands; the rest stream in behind it
CHUNKS = [1552] + [4608] * 7  # sums to XB
DELTA = 0.001
_GRID = (-1.0, 0.5, 2.0)


def _taps():
    """(tap_index, ky, kx, stream_delta) for the 9 taps."""
    out = []
    for ky in range(3):
        for kx in range(3):
            out.append((3 * ky + kx, ky, kx, ky * WP + (kx - 1)))
    return out


def build_kernel():
    nc = bacc.Bacc("TRN2", target_bir_lowering=False)
    x = nc.dram_tensor("x", [2 * C, XB], BF16, kind="ExternalInput")
    w = nc.dram_tensor("w", [O, 1152], BF16, kind="ExternalInput")
    y = nc.dram_tensor("y", [O, OL], BF16, kind="ExternalOutput")

    taps = _taps()

    with tile.TileContext(nc) as tc:
        with (
            tc.tile_pool(name="wg", bufs=1) as wg,
            tc.tile_pool(name="outp", bufs=3) as outp,
            tc.tile_pool(name="ps", bufs=2, space="PSUM") as ps,
        ):
            wt = wg.tile([O, 1152], BF16)
            # wt rides the sync ring so it transfers in parallel with the
            # first input chunk on the scalar ring
            nc.sync.dma_start(wt[:], w[:])
            xs = wg.tile([2 * C, XB], BF16)
            # input chunks ride the gpsimd SWDGE ring: the scalar and sync
            # queues then carry no input-DMA waits that could block the
            # PSUM->SBUF copies behind them mid-stream
            c0 = 0
            for clen in CHUNKS:
                nc.gpsimd.dma_start(
                    xs[:, c0 : c0 + clen], x[:, c0 : c0 + clen]
                )
                c0 += clen

            # Warm-up: ~10 matmuls on a never-written scratch tile. No data
            # deps, so they issue right after the preamble barrier and bring
            # the PE HAM clock-gate to 8/8 while the first input chunk DMA
            # is still in flight. Results land in a throwaway PSUM tile that
            # tag-rotates into the real pool afterwards.
            dmy = outp.tile([O, 512], BF16, tag="dmy", name="dmy")
            nc.vector.memset(dmy[:], 1.0)
            dmy_ps = ps.tile([O, 512], F32, tag="a0", name="dmy_ps")
            for _ in range(10):
                nc.tensor.matmul(
                    dmy_ps[:], dmy[0:O, 0:128], dmy[0:O, 0:512],
                    start=True, stop=True,
                )

            def emit_group(g, na, nb):
                pa = [
                    ps.tile([O, 512], F32, tag=f"a{j}", name=f"pa{j}")
                    for j in range(na)
                ]
                pb = [
                    ps.tile([O, 512], F32, tag=f"b{j}", name=f"pb{j}")
                    for j in range(nb)
                ]
                ntap = len(taps)
                for t, ky, kx, delta in taps:
                    lhs_a = wt[0:C, 128 * t : 128 * t + 128]
                    lhs_b = wt[C : 2 * C, 128 * t : 128 * t + 128]
                    first = t == 0
                    last = t == ntap - 1
                    for j in range(max(na, nb)):
                        o = 512 * (2 * g + j) + delta + GUARD
                        if j < na:
                            nc.tensor.matmul(
                                pa[j][:], lhs_a, xs[0:C, o : o + 512],
                                start=first, stop=last,
                            )
                        if j < nb:
                            nc.tensor.matmul(
                                pb[j][:], lhs_b, xs[C : 2 * C, o : o + 512],
                                start=first, stop=last,
                            )
                ot = outp.tile([O, 512 * (na + nb)], BF16, tag="ot", name="ot")
                for j in range(na):
                    if j % 2 == 0:
                        nc.scalar.copy(ot[:, 512 * j : 512 * j + 512], pa[j][:])
                    else:
                        nc.vector.tensor_copy(
                            ot[:, 512 * j : 512 * j + 512], pa[j][:]
                        )
                for j in range(nb):
                    c0 = 512 * (na + j)
                    if j % 2 == 1:
                        nc.scalar.copy(ot[:, c0 : c0 + 512], pb[j][:])
                    else:
                        nc.vector.tensor_copy(ot[:, c0 : c0 + 512], pb[j][:])
                if na:
                    nc.sync.dma_start(
                        y[:, 512 * 2 * g : 512 * (2 * g + na)],
                        ot[:, 0 : 512 * na],
                    )
                if nb:
                    nc.sync.dma_start(
                        y[:, 512 * (NW_A + 2 * g) : 512 * (NW_A + 2 * g + nb)],
                        ot[:, 512 * na : 512 * (na + nb)],
                    )

            for g in range(32):
                emit_group(g, 2, 2)
            # final window 128 (B-half local window 64)
            emit_group(32, 0, 1)

    nc.compile()
    return nc


_NC_CACHE = None


def _get_nc():
    global _NC_CACHE
    if _NC_CACHE is None:
        _NC_CACHE = build_kernel()
    return _NC_CACHE


def _host_weights(freq, theta, sigma, psi, f0, theta0, xg, yg):
    th = np.asarray(theta, np.float32)[:, :, None, None]
    sg = np.asarray(sigma, np.float32)[:, :, None, None]
    fr = np.asarray(freq, np.float32)[:, :, None, None]
    ps = np.asarray(psi, np.float32)[:, :, None, None]
    xg = np.asarray(xg, np.float32)
    yg = np.asarray(yg, np.float32)
    f0 = np.asarray(f0, np.float32)
    theta0 = np.asarray(theta0, np.float32)
    rotx = xg * np.cos(th) + yg * np.sin(th)
    roty = -xg * np.sin(th) + yg * np.cos(th)
    r = np.sqrt(rotx**2 + roty**2 + DELTA)
    g_rad = np.exp(-((np.log(r) - np.log(f0)) / (2.0 * np.log(sg / f0))) ** 2)
    g_ang = np.exp(-((th - theta0) ** 2) / (2.0 * sg**2))
    g = g_rad * g_ang * np.cos(fr * r + ps) / (2.0 * math.pi * sg**2)
    return g.astype(np.float32)  # [O, I, 3, 3]


def kernel(input_tensor, freq, theta, sigma, psi, f0, theta0, xg, yg):
    x = np.ascontiguousarray(np.asarray(input_tensor, dtype=np.float32))
    wfull = _host_weights(freq, theta, sigma, psi, f0, theta0, xg, yg)
    wt = np.zeros((O, 1152), np.float32)
    for t, ky, kx, _delta in _taps():
        blk = wfull[:, :, ky, kx].T  # lhsT [K=64, M=128]
        wt[0:C, 128 * t : 128 * t + 128] = blk
        wt[C : 2 * C, 128 * t : 128 * t + 128] = blk
    wt_bf = np.ascontiguousarray(wt.astype(BF16_NP))

    nc = _get_nc()
    in_maps = []
    for c in range(NB):
        s = np.zeros((C, H + 2, WP), np.float32)
        s[:, 1 : H + 1, 1 : W + 1] = x[c]
        s_bf = s.reshape(C, SL).astype(BF16_NP)
        xp = np.zeros((2 * C, XB), BF16_NP)
        xp[0:C, GUARD:XB] = s_bf[:, 0 : XB - GUARD]
        b0 = 512 * NW_A - GUARD
        nb_len = min(SL - b0, XB)
        xp[C : 2 * C, 0:nb_len] = s_bf[:, b0 : b0 + nb_len]
        in_maps.append({"x": xp, "w": wt_bf})
    res = run_bass_kernel_spmd(nc, in_maps, core_ids=list(range(NB)))
    out = np.empty((NB, O, H, W), np.float32)
    for c in range(NB):
        yv = np.asarray(res.results[c]["y"]).reshape(O, H, WP)
        out[c] = yv[:, :, 1 : W + 1].astype(np.float32)
    return out


# revision 9
# speedup vs baseline: 1.0689x; 1.0395x over previous
"""LogGaborConv2d on 8 TRN2 NeuronCores.

Strategy: data-parallel over batch (8 images -> 8 cores). Per core:
- Gabor weights [O=128, I=64, 3, 3] computed on host (73k elements),
  uploaded as bf16 lhsT blocks, duplicated across both PE row groups.
- 3x3 conv as 9 accumulating bf16 matmuls (K=64 input channels) over a
  column-padded flat image stream (width 258), windows of 512 pixels
  into PSUM banks. bf16 streams at 1 col/cycle (2x the fp32 rate) and
  enables fast weight load.
- The 128 PE rows are split into two row-groups: partitions 0:64
  process the top half of the image, partitions 64:128 the bottom half,
  as concurrent K=64 matmuls (tile_position row groups), doubling PE
  throughput vs a single K=64 stream.
- The whole per-core input stream (8.7 MB bf16) lives in SBUF: one
  persistent tile filled by 8 chunked DMAs on the scalar HWDGE ring,
  so input prefetch never queues behind output stores (sync ring).
- Output staged as bf16 (PSUM fp32 -> SBUF bf16 copies), halving store
  traffic; host upcasts to fp32.
"""
import math

import numpy as np
import ml_dtypes

import concourse.bacc as bacc
import concourse.bass as bass  # noqa: F401
import concourse.mybir as mybir
import concourse.tile as tile
from concourse.bass_utils import run_bass_kernel_spmd

F32 = mybir.dt.float32
BF16 = mybir.dt.bfloat16
BF16_NP = np.dtype(ml_dtypes.bfloat16)

# problem constants
NB, C, H, W = 8, 64, 256, 256
O = 128
WP = W + 2            # padded row width
SL = (H + 2) * WP     # padded input stream length (incl. top/bottom pad rows)
OL = H * WP           # padded output stream length
NWIN = OL // 512      # 129 windows of 512
GUARD = 4             # leading guard zeros in the device stream
NW_A = 64             # windows 0:64 on partitions 0:64; 64:129 on 64:128
XB = 33808            # per-half device stream cols (>= 512*65 + 517 + 4 + 511)
# first chunk covers exactly window group 0 (cols <= 1544) so the first
# matmul can start as soon as ~0.4 MB lands; the rest stream in behind it
# in ~0.6 MB pieces whose completion lag stays small
CHUNKS = [1552] + [2304] * 14  # sums to XB
DELTA = 0.001
_GRID = (-1.0, 0.5, 2.0)


def _taps():
    """(tap_index, ky, kx, stream_delta) for the 9 taps."""
    out = []
    for ky in range(3):
        for kx in range(3):
            out.append((3 * ky + kx, ky, kx, ky * WP + (kx - 1)))
    return out


def build_kernel():
    nc = bacc.Bacc("TRN2", target_bir_lowering=False)
    x = nc.dram_tensor("x", [2 * C, XB], BF16, kind="ExternalInput")
    w = nc.dram_tensor("w", [O, 1152], BF16, kind="ExternalInput")
    y = nc.dram_tensor("y", [O, OL], BF16, kind="ExternalOutput")

    taps = _taps()

    with tile.TileContext(nc) as tc:
        with (
            tc.tile_pool(name="wg", bufs=1) as wg,
            tc.tile_pool(name="outp", bufs=3) as outp,
            tc.tile_pool(name="ps", bufs=2, space="PSUM") as ps,
        ):
            wt = wg.tile([O, 1152], BF16)
            # wt rides the sync ring so it transfers in parallel with the
            # first input chunk on the scalar ring
            nc.sync.dma_start(wt[:], w[:])
            xs = wg.tile([2 * C, XB], BF16)
            # input chunks ride the gpsimd SWDGE ring: the scalar and sync
            # queues then carry no input-DMA waits that could block the
            # PSUM->SBUF copies behind them mid-stream
            c0 = 0
            for clen in CHUNKS:
                nc.gpsimd.dma_start(
                    xs[:, c0 : c0 + clen], x[:, c0 : c0 + clen]
                )
                c0 += clen

            # Warm-up: ~10 matmuls on a never-written scratch tile. No data
            # deps, so they issue right after the preamble barrier and bring
            # the PE HAM clock-gate to 8/8 while the first input chunk DMA
            # is still in flight. Results land in a throwaway PSUM tile that
            # tag-rotates into the real pool afterwards.
            dmy = outp.tile([O, 512], BF16, tag="dmy", name="dmy")
            nc.vector.memset(dmy[:], 1.0)
            dmy_ps = ps.tile([O, 512], F32, tag="a0", name="dmy_ps")
            for _ in range(10):
                nc.tensor.matmul(
                    dmy_ps[:], dmy[0:O, 0:128], dmy[0:O, 0:512],
                    start=True, stop=True,
                )

            def emit_group(g, na, nb):
                pa = [
                    ps.tile([O, 512], F32, tag=f"a{j}", name=f"pa{j}")
                    for j in range(na)
                ]
                pb = [
                    ps.tile([O, 512], F32, tag=f"b{j}", name=f"pb{j}")
                    for j in range(nb)
                ]
                ntap = len(taps)
                for t, ky, kx, delta in taps:
                    lhs_a = wt[0:C, 128 * t : 128 * t + 128]
                    lhs_b = wt[C : 2 * C, 128 * t : 128 * t + 128]
                    first = t == 0
                    last = t == ntap - 1
                    for j in range(max(na, nb)):
                        o = 512 * (2 * g + j) + delta + GUARD
                        if j < na:
                            nc.tensor.matmul(
                                pa[j][:], lhs_a, xs[0:C, o : o + 512],
                                start=first, stop=last,
                            )
                        if j < nb:
                            nc.tensor.matmul(
                                pb[j][:], lhs_b, xs[C : 2 * C, o : o + 512],
                                start=first, stop=last,
                            )
                ot = outp.tile([O, 512 * (na + nb)], BF16, tag="ot", name="ot")
                for j in range(na):
                    if j % 2 == 0:
                        nc.scalar.copy(ot[:, 512 * j : 512 * j + 512], pa[j][:])
                    else:
                        nc.vector.tensor_copy(
                            ot[:, 512 * j : 512 * j + 512], pa[j][:]
                        )
                for j in range(nb):
                    c0 = 512 * (na + j)
                    if j % 2 == 1:
                        nc.scalar.copy(ot[:, c0 : c0 + 512], pb[j][:])
                    else:
                        nc.vector.tensor_copy(ot[:, c0 : c0 + 512], pb[j][:])
                if na:
                    nc.sync.dma_start(
                        y[:, 512 * 2 * g : 512 * (2 * g + na)],
                        ot[:, 0 : 512 * na],
                    )
                if nb:
                    nc.sync.dma_start(
                        y[:, 512 * (NW_A + 2 * g) : 512 * (NW_A + 2 * g + nb)],
                        ot[:, 512 * na : 512 * (na + nb)],
                    )

            for g in range(32):
                emit_group(g, 2, 2)
            # final window 128 (B-half local window 64)
            emit_group(32, 0, 1)

    nc.compile()
    return nc


_NC_CACHE = None


def _get_nc():
    global _NC_CACHE
    if _NC_CACHE is None:
        _NC_CACHE = build_kernel()
    return _NC_CACHE


def _host_weights(freq, theta, sigma, psi, f0, theta0, xg, yg):
    th = np.asarray(theta, np.float32)[:, :, None, None]
    sg = np.asarray(sigma, np.float32)[:, :, None, None]
    fr = np.asarray(freq, np.float32)[:, :, None, None]
    ps = np.asarray(psi, np.float32)[:, :, None, None]
    xg = np.asarray(xg, np.float32)
    yg = np.asarray(yg, np.float32)
    f0 = np.asarray(f0, np.float32)
    theta0 = np.asarray(theta0, np.float32)
    rotx = xg * np.cos(th) + yg * np.sin(th)
    roty = -xg * np.sin(th) + yg * np.cos(th)
    r = np.sqrt(rotx**2 + roty**2 + DELTA)
    g_rad = np.exp(-((np.log(r) - np.log(f0)) / (2.0 * np.log(sg / f0))) ** 2)
    g_ang = np.exp(-((th - theta0) ** 2) / (2.0 * sg**2))
    g = g_rad * g_ang * np.cos(fr * r + ps) / (2.0 * math.pi * sg**2)
    return g.astype(np.float32)  # [O, I, 3, 3]


def kernel(input_tensor, freq, theta, sigma, psi, f0, theta0, xg, yg):
    x = np.ascontiguousarray(np.asarray(input_tensor, dtype=np.float32))
    wfull = _host_weights(freq, theta, sigma, psi, f0, theta0, xg, yg)
    wt = np.zeros((O, 1152), np.float32)
    for t, ky, kx, _delta in _taps():
        blk = wfull[:, :, ky, kx].T  # lhsT [K=64, M=128]
        wt[0:C, 128 * t : 128 * t + 128] = blk
        wt[C : 2 * C, 128 * t : 128 * t + 128] = blk
    wt_bf = np.ascontiguousarray(wt.astype(BF16_NP))

    nc = _get_nc()
    in_maps = []
    for c in range(NB):
        s = np.zeros((C, H + 2, WP), np.float32)
        s[:, 1 : H + 1, 1 : W + 1] = x[c]
        s_bf = s.reshape(C, SL).astype(BF16_NP)
        xp = np.zeros((2 * C, XB), BF16_NP)
        xp[0:C, GUARD:XB] = s_bf[:, 0 : XB - GUARD]
        b0 = 512 * NW_A - GUARD
        nb_len = min(SL - b0, XB)
        xp[C : 2 * C, 0:nb_len] = s_bf[:, b0 : b0 + nb_len]
        in_maps.append({"x": xp, "w": wt_bf})
    res = run_bass_kernel_spmd(nc, in_maps, core_ids=list(range(NB)))
    out = np.empty((NB, O, H, W), np.float32)
    for c in range(NB):
        yv = np.asarray(res.results[c]["y"]).reshape(O, H, WP)
        out[c] = yv[:, :, 1 : W + 1].astype(np.float32)
    return out
